# revision 1
# baseline (speedup 1.0000x reference)
"""Trainium2 Bass kernel for complex depthwise batchnorm (training-mode stats).

Data-parallel over batch N across 8 NeuronCores. Per core:
  phase A: stream the [2048, 2056] shard, accumulate per-column
           {sum xr, sum xi, sum xr^2, sum xi^2, sum xr*xi} via ones-vector
           matmuls into PSUM (fp32 matmuls for plain sums, bf16 for the
           three product sums — squares/cross are written bf16 by ACT/DVE).
  AllReduce (41KB) of the 5x2056 sums across cores.
  coefficient math on [8, 257]: 2x2 inverse-sqrt covariance whitening +
           affine mixing collapsed to y = Z@x + b' per column.
  phase B: stream the shard again, yr = Zrr*xr + Zri*xi + br',
           yi = Zir*xr + Zii*xi + bi' with coefficients broadcast across
           partitions via PE ones-broadcast; ops split across DVE + GpSimd.
"""

import numpy as np

N, C, F = 16384, 8, 257
D = C * F  # 2056
N_CORES = 8
NS = N // N_CORES  # 2048
P = 128
T = NS // P  # 16 tiles per core
EPS = 1e-6
DELTA_MAX = 1e8

# free-dim chunks for reduction matmuls (PSUM bank = 512 fp32)
# 4 full 512-wide chunks cover [0, 2048); the 8-col tail is packed separately
RED_CHUNKS = [(c * 512, 512) for c in range(4)]
TAIL_OFF, TAIL_W = 2048, D - 2048  # 8 columns
# column halves for phase B elementwise work
HALVES = [(0, D // 2), (D // 2, D - D // 2)]

_CACHE = {}


def _build():
    import concourse.bacc as bacc
    import concourse.tile as tile
    import concourse.mybir as mybir

    f32 = mybir.dt.float32
    bf16 = mybir.dt.bfloat16
    Alu = mybir.AluOpType
    Act = mybir.ActivationFunctionType

    nc = bacc.Bacc("TRN2", target_bir_lowering=False, debug=False,
                   num_devices=N_CORES)

    xr = nc.dram_tensor("xr", [NS, D], f32, kind="ExternalInput").ap()
    xi = nc.dram_tensor("xi", [NS, D], f32, kind="ExternalInput").ap()
    wrr = nc.dram_tensor("wrr", [C, F], f32, kind="ExternalInput").ap()
    wri = nc.dram_tensor("wri", [C, F], f32, kind="ExternalInput").ap()
    wii = nc.dram_tensor("wii", [C, F], f32, kind="ExternalInput").ap()
    br = nc.dram_tensor("br", [C, F], f32, kind="ExternalInput").ap()
    bi = nc.dram_tensor("bi", [C, F], f32, kind="ExternalInput").ap()
    yr = nc.dram_tensor("yr", [NS, D], f32, kind="ExternalOutput").ap()
    yi = nc.dram_tensor("yi", [NS, D], f32, kind="ExternalOutput").ap()

    with tile.TileContext(nc) as tc:
        with (
            tc.tile_pool(name="const", bufs=1) as cpool,
            tc.tile_pool(name="inp", bufs=3) as inp,
            tc.tile_pool(name="sq", bufs=8) as sqp,
            tc.tile_pool(name="tmpb", bufs=2) as tmpb,
            tc.tile_pool(name="small", bufs=1) as smp,
            tc.tile_pool(name="ctmp", bufs=6) as ctp,
            tc.tile_pool(name="dram", bufs=1, space="DRAM") as dram,
        ):
            ones_f = cpool.tile([P, 1], f32, name="ones_f")
            nc.vector.memset(ones_f[:], 1.0)
            ones_b = cpool.tile([P, 1], bf16, name="ones_b")
            nc.vector.memset(ones_b[:], 1.0)
            ones_row = cpool.tile([1, P], f32, name="ones_row")
            nc.vector.memset(ones_row[:], 1.0)

            # ---------------- phase A: local sums ----------------
            # PE matmul outputs must start at partition 0/32/64. Quantities:
            #   accA: q0=sum(xr)@p0, q1=sum(xi)@p32, q2=sum(xr^2)@p64
            #   accB: q3=sum(xi^2)@p0, q4=sum(xr*xi)@p32,
            #         all 5 tails (cols 2048:2056) @p64, free offset q*16
            cc_in = dram.tile([5, D], f32, name="cc_in")
            cc_out = dram.tile([5, D], f32, name="cc_out", addr_space="Shared")
            with tc.tile_pool(name="acc", bufs=1, space="PSUM") as accp:
                accA = accp.tile([65, 2048], f32, name="accA")  # 4 banks
                accB = accp.tile([65, 2048], f32, name="accB")  # 4 banks
                # (tile, base partition, tail free offset) per quantity
                QSLOT = [(accA, 0), (accA, 32), (accA, 64),
                         (accB, 0), (accB, 32)]

                for i in range(T):
                    xr_t = inp.tile([P, D], f32, tag="xr", name=f"xr_{i}")
                    nc.sync.dma_start(out=xr_t[:], in_=xr[i * P:(i + 1) * P, :])
                    xi_t = inp.tile([P, D], f32, tag="xi", name=f"xi_{i}")
                    nc.sync.dma_start(out=xi_t[:], in_=xi[i * P:(i + 1) * P, :])

                    st, fin = (i == 0), (i == T - 1)

                    def red(q, rhs_ap, ones_t):
                        tile_, p = QSLOT[q]
                        off, w = CUR_CHUNK
                        if off < TAIL_OFF:
                            nc.tensor.matmul(tile_[p:p + 1, off:off + w],
                                             lhsT=ones_t[:], rhs=rhs_ap,
                                             start=st, stop=fin)
                        else:
                            # all 5 tails share one 2KB zero region at
                            # accB partition 64: q0's first matmul zeroes it,
                            # q4's last matmul closes the group
                            nc.tensor.matmul(accB[64:65, q * 8:q * 8 + w],
                                             lhsT=ones_t[:], rhs=rhs_ap,
                                             start=(st and q == 0),
                                             stop=(fin and q == 4))

                    for off, w in RED_CHUNKS + [(TAIL_OFF, TAIL_W)]:
                        CUR_CHUNK = (off, w)
                        sl = slice(off, off + w)
                        red(0, xr_t[:, sl], ones_f)
                        red(1, xi_t[:, sl], ones_f)
                        sqr = sqp.tile([P, 512], bf16, tag="sqr",
                                       name=f"sqr_{i}_{off}")
                        nc.scalar.activation(sqr[:, 0:w], xr_t[:, sl],
                                             Act.Square)
                        red(2, sqr[:, 0:w], ones_b)
                        sqi = sqp.tile([P, 512], bf16, tag="sqi",
                                       name=f"sqi_{i}_{off}")
                        nc.scalar.activation(sqi[:, 0:w], xi_t[:, sl],
                                             Act.Square)
                        red(3, sqi[:, 0:w], ones_b)
                        crs = sqp.tile([P, 512], bf16, tag="crs",
                                       name=f"crs_{i}_{off}")
                        nc.vector.tensor_tensor(crs[:, 0:w], xr_t[:, sl],
                                                xi_t[:, sl], Alu.mult)
                        red(4, crs[:, 0:w], ones_b)

                # partition-aligned PSUM -> SBUF copies, then row-gather DMAs
                # (staged in the phase-B temp slots, idle at this point)
                sums_a = tmpb.tile([65, 2048], f32, tag="t1", name="sums_a")
                sums_b = tmpb.tile([65, 2048], f32, tag="t2", name="sums_b")
                nc.vector.tensor_copy(sums_a[0:1, :], accA[0:1, :])
                nc.scalar.copy(sums_a[32:33, :], accA[32:33, :])
                nc.vector.tensor_copy(sums_a[64:65, :], accA[64:65, :])
                nc.scalar.copy(sums_b[0:1, :], accB[0:1, :])
                nc.vector.tensor_copy(sums_b[32:33, :], accB[32:33, :])
                nc.scalar.copy(sums_b[64:65, 0:40], accB[64:65, 0:40])

            SB_SLOT = [(sums_a, 0), (sums_a, 32), (sums_a, 64),
                       (sums_b, 0), (sums_b, 32)]
            for q, (tile_, p) in enumerate(SB_SLOT):
                nc.sync.dma_start(out=cc_in[q:q + 1, 0:TAIL_OFF],
                                  in_=tile_[p:p + 1, :])
                nc.sync.dma_start(
                    out=cc_in[q:q + 1, TAIL_OFF:D],
                    in_=sums_b[64:65, q * 8:q * 8 + TAIL_W])

            # ---------------- all-reduce ----------------
            nc.gpsimd.collective_compute(
                "AllReduce",
                Alu.add,
                replica_groups=[list(range(N_CORES))],
                ins=[cc_in[:].opt()],
                outs=[cc_out[:].opt()],
            )
            cc_cf = cc_out[:].rearrange("q (c f) -> (q c) f", c=C)

            def load_cf(name, src):
                t = smp.tile([C, F], f32, name=name)
                nc.sync.dma_start(out=t[:], in_=src)
                return t

            s_xr = load_cf("s_xr", cc_cf[0 * C:1 * C, :])
            s_xi = load_cf("s_xi", cc_cf[1 * C:2 * C, :])
            s_rr = load_cf("s_rr", cc_cf[2 * C:3 * C, :])
            s_ii = load_cf("s_ii", cc_cf[3 * C:4 * C, :])
            s_ri = load_cf("s_ri", cc_cf[4 * C:5 * C, :])
            w_rr = load_cf("w_rr", wrr[:, :])
            w_ri = load_cf("w_ri", wri[:, :])
            w_ii = load_cf("w_ii", wii[:, :])
            b_r = load_cf("b_r", br[:, :])
            b_i = load_cf("b_i", bi[:, :])

            # ---------------- coefficient math on [C, F] ----------------
            inv_n = 1.0 / N
            V = nc.vector
            S = nc.scalar

            def keep(name):
                return smp.tile([C, F], f32, name=name)

            def scratch(name):
                return ctp.tile([C, F], f32, tag="ct", name=name)

            mr = keep("mr")
            V.tensor_scalar_mul(mr[:], s_xr[:], inv_n)
            mi = keep("mi")
            V.tensor_scalar_mul(mi[:], s_xi[:], inv_n)

            mr2 = scratch("mr2")
            V.tensor_tensor(mr2[:], mr[:], mr[:], Alu.mult)
            mi2 = scratch("mi2")
            V.tensor_tensor(mi2[:], mi[:], mi[:], Alu.mult)
            mri = scratch("mri")
            V.tensor_tensor(mri[:], mr[:], mi[:], Alu.mult)

            vrr = keep("vrr")
            V.scalar_tensor_tensor(vrr[:], s_rr[:], inv_n, mr2[:],
                                   Alu.mult, Alu.subtract)
            vii = keep("vii")
            V.scalar_tensor_tensor(vii[:], s_ii[:], inv_n, mi2[:],
                                   Alu.mult, Alu.subtract)
            vri = keep("vri")
            V.scalar_tensor_tensor(vri[:], s_ri[:], inv_n, mri[:],
                                   Alu.mult, Alu.subtract)

            tau = keep("tau")
            V.tensor_tensor(tau[:], vrr[:], vii[:], Alu.add)
            d1 = scratch("d1")
            V.tensor_tensor(d1[:], vrr[:], vii[:], Alu.mult)
            vri2 = scratch("vri2")
            V.tensor_tensor(vri2[:], vri[:], vri[:], Alu.mult)
            delta = keep("delta")
            V.tensor_tensor(delta[:], d1[:], vri2[:], Alu.subtract)
            V.tensor_scalar_max(delta[:], delta[:], EPS)
            V.tensor_scalar_min(delta[:], delta[:], DELTA_MAX)

            s_t = keep("s_t")
            S.activation(s_t[:], delta[:], Act.Sqrt)
            targ = scratch("targ")
            V.scalar_tensor_tensor(targ[:], s_t[:], 2.0, tau[:],
                                   Alu.mult, Alu.add)
            t_t = keep("t_t")
            S.activation(t_t[:], targ[:], Act.Sqrt)
            st_t = scratch("st_t")
            V.tensor_tensor(st_t[:], s_t[:], t_t[:], Alu.mult)
            rst = keep("rst")
            V.reciprocal(rst[:], st_t[:])

            a1 = scratch("a1")
            V.tensor_tensor(a1[:], s_t[:], vii[:], Alu.add)
            urr = keep("urr")
            V.tensor_tensor(urr[:], a1[:], rst[:], Alu.mult)
            a2 = scratch("a2")
            V.tensor_tensor(a2[:], s_t[:], vrr[:], Alu.add)
            uii = keep("uii")
            V.tensor_tensor(uii[:], a2[:], rst[:], Alu.mult)
            uri = keep("uri")
            V.scalar_tensor_tensor(uri[:], vri[:], -1.0, rst[:],
                                   Alu.mult, Alu.mult)

            def mix(name, wa, ua, wb, ub):
                g1 = scratch(name + "_g1")
                V.tensor_tensor(g1[:], wa[:], ua[:], Alu.mult)
                g2 = scratch(name + "_g2")
                V.tensor_tensor(g2[:], wb[:], ub[:], Alu.mult)
                z = keep(name)
                V.tensor_tensor(z[:], g1[:], g2[:], Alu.add)
                return z

            zrr = mix("zrr", w_rr, urr, w_ri, uri)
            zri = mix("zri", w_rr, uri, w_ri, uii)
            zir = mix("zir", w_ri, urr, w_ii, uri)
            zii = mix("zii", w_ri, uri, w_ii, uii)

            def bias(name, b0, za, zb):
                h1 = scratch(name + "_h1")
                V.tensor_tensor(h1[:], za[:], mr[:], Alu.mult)
                h2 = scratch(name + "_h2")
                V.tensor_tensor(h2[:], zb[:], mi[:], Alu.mult)
                h3 = scratch(name + "_h3")
                V.tensor_tensor(h3[:], h1[:], h2[:], Alu.add)
                bb = keep(name)
                V.tensor_tensor(bb[:], b0[:], h3[:], Alu.subtract)
                return bb

            brp = bias("brp", b_r, zrr, zri)
            bip = bias("bip", b_i, zir, zii)

            # ---------------- broadcast coeffs to [128, D] ----------------
            # repack each [C, F] coeff into a [1, D] partition-0 row (DMA),
            # then PE ones-broadcast (matmul rhs must sit at partition 0)
            bcs = []
            with tc.tile_pool(name="bps", bufs=4, space="PSUM") as bps:
                for k, coef in enumerate([zrr, zri, zir, zii, brp, bip]):
                    row = smp.tile([1, D], f32, tag="row", name=f"row{k}")
                    nc.sync.dma_start(out=row[0:1, :], in_=coef[:])
                    bc = cpool.tile([P, D], f32, name=f"bc{k}")
                    for off, w in RED_CHUNKS + [(TAIL_OFF, TAIL_W)]:
                        pb = bps.tile([P, 512], f32, tag="pb",
                                      name=f"pb{k}_{off}")
                        nc.tensor.matmul(pb[:, 0:w], lhsT=ones_row[:],
                                         rhs=row[0:1, off:off + w],
                                         start=True, stop=True)
                        nc.scalar.copy(bc[:, off:off + w], pb[:, 0:w])
                    bcs.append(bc)
            bzrr, bzri, bzir, bzii, bbrp, bbip = bcs

            # ---------------- phase B: apply ----------------
            for i in range(T):
                xr_t = inp.tile([P, D], f32, tag="xr", name=f"xr2_{i}")
                nc.sync.dma_start(out=xr_t[:], in_=xr[i * P:(i + 1) * P, :])
                xi_t = inp.tile([P, D], f32, tag="xi", name=f"xi2_{i}")
                nc.sync.dma_start(out=xi_t[:], in_=xi[i * P:(i + 1) * P, :])

                t1 = tmpb.tile([P, D], f32, tag="t1", name=f"t1_{i}")
                t2 = tmpb.tile([P, D], f32, tag="t2", name=f"t2_{i}")

                # full-width ops; xr_t/xi_t overwritten in place after reads.
                # 6 ops on DVE, 2 on GpSimd (POOL is ~2.4x slower per element
                # and shares SBUF ports with DVE).
                nc.vector.tensor_tensor(t1[:], xr_t[:], bzrr[:], Alu.mult)
                nc.gpsimd.tensor_tensor(t2[:], xi_t[:], bzri[:], Alu.mult)
                nc.gpsimd.tensor_tensor(xr_t[:], xr_t[:], bzir[:], Alu.mult)
                nc.vector.tensor_tensor(xi_t[:], xi_t[:], bzii[:], Alu.mult)
                # yr = t1 + t2 + brp
                nc.vector.tensor_tensor(t1[:], t1[:], t2[:], Alu.add)
                nc.vector.tensor_tensor(t1[:], t1[:], bbrp[:], Alu.add)
                # yi = xr_t + xi_t + bip
                nc.vector.tensor_tensor(xr_t[:], xr_t[:], xi_t[:], Alu.add)
                nc.vector.tensor_tensor(xr_t[:], xr_t[:], bbip[:], Alu.add)
                nc.sync.dma_start(out=yr[i * P:(i + 1) * P, :], in_=t1[:])
                nc.sync.dma_start(out=yi[i * P:(i + 1) * P, :], in_=xr_t[:])

    nc.compile()
    return nc


def get_nc():
    if "nc" not in _CACHE:
        _CACHE["nc"] = _build()
    return _CACHE["nc"]


def kernel(xr, xi, Wrr, Wri, Wii, Br, Bi):
    from concourse import bass_utils

    nc = get_nc()
    xr2 = np.ascontiguousarray(np.asarray(xr), dtype=np.float32).reshape(N, D)
    xi2 = np.ascontiguousarray(np.asarray(xi), dtype=np.float32).reshape(N, D)
    params = {
        "wrr": np.ascontiguousarray(np.asarray(Wrr), dtype=np.float32),
        "wri": np.ascontiguousarray(np.asarray(Wri), dtype=np.float32),
        "wii": np.ascontiguousarray(np.asarray(Wii), dtype=np.float32),
        "br": np.ascontiguousarray(np.asarray(Br), dtype=np.float32),
        "bi": np.ascontiguousarray(np.asarray(Bi), dtype=np.float32),
    }
    in_maps = []
    for r in range(N_CORES):
        m = {"xr": xr2[r * NS:(r + 1) * NS], "xi": xi2[r * NS:(r + 1) * NS]}
        m.update(params)
        in_maps.append(m)

    res = bass_utils.run_bass_kernel_spmd(nc, in_maps,
                                          core_ids=list(range(N_CORES)))
    yr_ = np.concatenate([res.results[r]["yr"] for r in range(N_CORES)], axis=0)
    yi_ = np.concatenate([res.results[r]["yi"] for r in range(N_CORES)], axis=0)
    return yr_.reshape(N, C, F), yi_.reshape(N, C, F)



# revision 7
# speedup vs baseline: 1.5961x; 1.5961x over previous
"""Trainium2 Bass kernel for complex depthwise batchnorm (training-mode stats).

Data-parallel over batch N across 8 NeuronCores. Per core:
  phase A: stream the [2048, 2056] fp32 shard once; cast to a bf16 SBUF
           cache (DVE 2x copy), form squares (ACT) / cross (DVE) in bf16,
           and accumulate the 5 per-column sums {xr, xi, xr^2, xi^2, xr*xi}
           via ones-vector matmuls into PSUM.
  AllReduce (41KB) of the 5x2056 sums across cores.
  coefficient math on [8, 257]: 2x2 inverse-sqrt covariance whitening +
           affine mixing collapsed to y = Z@x + b' per column. The 4 Z
           coefficients are PE-broadcast to [128, 2056] bf16; the 2 biases
           stay as [1, 2056] bf16 rows.
  phase B: read the bf16 cache (no HBM re-read). DVE forms the 4 products
           z*x in bf16 2x mode; PE sums them pairwise in PSUM via
           identity-matmuls, seeding each accumulation with ones x bias_row
           outer products; ACT drains PSUM -> fp32 SBUF; DMA out.
"""

import numpy as np

N, C, F = 16384, 8, 257
D = C * F  # 2056
N_CORES = 8
NS = N // N_CORES  # 2048
P = 128
T = NS // P  # 16 tiles per core
EPS = 1e-6
DELTA_MAX = 1e8

# free-dim chunks for PSUM-bank-aligned matmuls
RED_CHUNKS = [(c * 512, 512) for c in range(4)]
TAIL_OFF, TAIL_W = 2048, D - 2048  # 8 columns

_CACHE = {}


def _build():
    import concourse.bacc as bacc
    import concourse.tile as tile
    import concourse.mybir as mybir

    f32 = mybir.dt.float32
    bf16 = mybir.dt.bfloat16
    Alu = mybir.AluOpType
    Act = mybir.ActivationFunctionType

    nc = bacc.Bacc("TRN2", target_bir_lowering=False, debug=False,
                   num_devices=N_CORES)

    xr = nc.dram_tensor("xr", [NS, D], f32, kind="ExternalInput").ap()
    xi = nc.dram_tensor("xi", [NS, D], f32, kind="ExternalInput").ap()
    wrr = nc.dram_tensor("wrr", [C, F], f32, kind="ExternalInput").ap()
    wri = nc.dram_tensor("wri", [C, F], f32, kind="ExternalInput").ap()
    wii = nc.dram_tensor("wii", [C, F], f32, kind="ExternalInput").ap()
    br = nc.dram_tensor("br", [C, F], f32, kind="ExternalInput").ap()
    bi = nc.dram_tensor("bi", [C, F], f32, kind="ExternalInput").ap()
    yr = nc.dram_tensor("yr", [NS, D], f32, kind="ExternalOutput").ap()
    yi = nc.dram_tensor("yi", [NS, D], f32, kind="ExternalOutput").ap()

    ALL_CHUNKS = RED_CHUNKS + [(TAIL_OFF, TAIL_W)]

    with tile.TileContext(nc) as tc:
        with (
            tc.tile_pool(name="const", bufs=1) as cpool,
            tc.tile_pool(name="cache", bufs=1) as cachep,
            tc.tile_pool(name="dram", bufs=1, space="DRAM") as dram,
        ):
            ones_b = cpool.tile([P, 1], bf16, name="ones_b")
            nc.vector.memset(ones_b[:], 1.0)
            ones_row = cpool.tile([1, P], bf16, name="ones_row")
            nc.vector.memset(ones_row[:], 1.0)
            # 128x128 bf16 identity for PE pass-through adds
            ident = cpool.tile([P, P], bf16, name="ident")
            nc.vector.memset(ident[:], 1.0)
            nc.gpsimd.affine_select(
                out=ident[:], in_=ident[:],
                compare_op=Alu.is_equal, fill=0.0,
                base=0, pattern=[[-1, P]], channel_multiplier=1,
            )

            # bf16 cache of the full shard (lives for the whole kernel)
            crs_tiles = [cachep.tile([P, D], bf16, name=f"cr{i}")
                         for i in range(T)]
            cis_tiles = [cachep.tile([P, D], bf16, name=f"ci{i}")
                         for i in range(T)]

            cc_in = dram.tile([5, D], f32, name="cc_in")
            cc_out = dram.tile([5, D], f32, name="cc_out", addr_space="Shared")

            # ---------------- phase A: local sums + bf16 cache ----------
            # PE matmul outputs must start at partition 0/32/64. Quantities:
            #   accA: q0=sum(xr)@p0, q1=sum(xi)@p32, q2=sum(xr^2)@p64
            #   accB: q3=sum(xi^2)@p0, q4=sum(xr*xi)@p32,
            #         all 5 tails (cols 2048:2056) @p64, free offset q*8
            with (
                tc.tile_pool(name="inpA", bufs=2) as inp,
                tc.tile_pool(name="sqA", bufs=2) as sqp,
                tc.tile_pool(name="accA", bufs=1, space="PSUM") as accp,
            ):
                accA = accp.tile([65, 2048], f32, name="accA")  # 4 banks
                accB = accp.tile([65, 2048], f32, name="accB")  # 4 banks
                QSLOT = [(accA, 0), (accA, 32), (accA, 64),
                         (accB, 0), (accB, 32)]

                for i in range(T):
                    xr_t = inp.tile([P, D], f32, tag="xr", name=f"xr_{i}")
                    nc.sync.dma_start(out=xr_t[:], in_=xr[i * P:(i + 1) * P, :])
                    xi_t = inp.tile([P, D], f32, tag="xi", name=f"xi_{i}")
                    nc.sync.dma_start(out=xi_t[:], in_=xi[i * P:(i + 1) * P, :])

                    cr = crs_tiles[i]
                    ci = cis_tiles[i]
                    # fp32 -> bf16 casts on DVE (2x_2P single-src mode)
                    nc.vector.tensor_copy(cr[:], xr_t[:])
                    nc.vector.tensor_copy(ci[:], xi_t[:])
                    # squares on ACT (dtype-independent 1x), cross on DVE
                    sqr = sqp.tile([P, D], bf16, tag="sqr", name=f"sqr_{i}")
                    nc.scalar.activation(sqr[:], xr_t[:], Act.Square)
                    sqi = sqp.tile([P, D], bf16, tag="sqi", name=f"sqi_{i}")
                    nc.scalar.activation(sqi[:], xi_t[:], Act.Square)
                    crs = sqp.tile([P, D], bf16, tag="crs", name=f"crs_{i}")
                    nc.vector.tensor_tensor(crs[:], cr[:], ci[:], Alu.mult)

                    st, fin = (i == 0), (i == T - 1)
                    RHS = [cr, ci, sqr, sqi, crs]
                    for off, w in RED_CHUNKS:
                        sl = slice(off, off + w)
                        for q in range(5):
                            tile_, p = QSLOT[q]
                            nc.tensor.matmul(tile_[p:p + 1, sl],
                                             lhsT=ones_b[:], rhs=RHS[q][:, sl],
                                             start=st, stop=fin)
                    # all 5 tails share one zero region at accB partition 64:
                    # q0's first matmul opens the group, q4's last closes it
                    for q in range(5):
                        nc.tensor.matmul(
                            accB[64:65, q * 8:q * 8 + TAIL_W],
                            lhsT=ones_b[:],
                            rhs=RHS[q][:, TAIL_OFF:D],
                            start=(st and q == 0), stop=(fin and q == 4))

                # partition-aligned PSUM -> SBUF copies, then row-gather DMAs
                sums_a = inp.tile([P, D], f32, tag="xr", name="sums_a")
                sums_b = inp.tile([P, D], f32, tag="xi", name="sums_b")
                nc.vector.tensor_copy(sums_a[0:1, 0:2048], accA[0:1, :])
                nc.scalar.copy(sums_a[32:33, 0:2048], accA[32:33, :])
                nc.vector.tensor_copy(sums_a[64:65, 0:2048], accA[64:65, :])
                nc.scalar.copy(sums_b[0:1, 0:2048], accB[0:1, :])
                nc.vector.tensor_copy(sums_b[32:33, 0:2048], accB[32:33, :])
                nc.scalar.copy(sums_b[64:65, 0:40], accB[64:65, 0:40])

                SB_SLOT = [(sums_a, 0), (sums_a, 32), (sums_a, 64),
                           (sums_b, 0), (sums_b, 32)]
                for q, (tile_, p) in enumerate(SB_SLOT):
                    nc.sync.dma_start(out=cc_in[q:q + 1, 0:TAIL_OFF],
                                      in_=tile_[p:p + 1, 0:2048])
                    nc.sync.dma_start(
                        out=cc_in[q:q + 1, TAIL_OFF:D],
                        in_=sums_b[64:65, q * 8:q * 8 + TAIL_W])

            # ---------------- all-reduce ----------------
            nc.gpsimd.collective_compute(
                "AllReduce",
                Alu.add,
                replica_groups=[list(range(N_CORES))],
                ins=[cc_in[:].opt()],
                outs=[cc_out[:].opt()],
            )
            cc_cf = cc_out[:].rearrange("q (c f) -> (q c) f", c=C)

            # allocated after phase A's pools are released (SBUF budget)
            from contextlib import ExitStack
            lateps = ExitStack()
            rowsk = lateps.enter_context(tc.tile_pool(name="rowsk", bufs=1))
            zbp = lateps.enter_context(tc.tile_pool(name="zb", bufs=1))
            # bias rows persist into phase B
            brow = rowsk.tile([1, D], bf16, name="brow")
            birow = rowsk.tile([1, D], bf16, name="birow")
            # Z coefficient broadcasts persist into phase B
            zb = [zbp.tile([P, D], bf16, name=f"zb{k}") for k in range(4)]

            with (
                tc.tile_pool(name="mid", bufs=1) as smp,
                tc.tile_pool(name="mids", bufs=4) as ctp,
                tc.tile_pool(name="bps", bufs=4, space="PSUM") as bps,
            ):
                def load_cf(name, src):
                    t = smp.tile([C, F], f32, name=name)
                    nc.sync.dma_start(out=t[:], in_=src)
                    return t

                s_xr = load_cf("s_xr", cc_cf[0 * C:1 * C, :])
                s_xi = load_cf("s_xi", cc_cf[1 * C:2 * C, :])
                s_rr = load_cf("s_rr", cc_cf[2 * C:3 * C, :])
                s_ii = load_cf("s_ii", cc_cf[3 * C:4 * C, :])
                s_ri = load_cf("s_ri", cc_cf[4 * C:5 * C, :])
                w_rr = load_cf("w_rr", wrr[:, :])
                w_ri = load_cf("w_ri", wri[:, :])
                w_ii = load_cf("w_ii", wii[:, :])
                b_r = load_cf("b_r", br[:, :])
                b_i = load_cf("b_i", bi[:, :])

                inv_n = 1.0 / N
                V = nc.vector
                S = nc.scalar

                def keep(name):
                    return smp.tile([C, F], f32, name=name)

                def scratch(name):
                    return ctp.tile([C, F], f32, tag="ct", name=name)

                mr = keep("mr")
                V.tensor_scalar_mul(mr[:], s_xr[:], inv_n)
                mi = keep("mi")
                V.tensor_scalar_mul(mi[:], s_xi[:], inv_n)

                mr2 = scratch("mr2")
                V.tensor_tensor(mr2[:], mr[:], mr[:], Alu.mult)
                mi2 = scratch("mi2")
                V.tensor_tensor(mi2[:], mi[:], mi[:], Alu.mult)
                mri = scratch("mri")
                V.tensor_tensor(mri[:], mr[:], mi[:], Alu.mult)

                vrr = keep("vrr")
                V.scalar_tensor_tensor(vrr[:], s_rr[:], inv_n, mr2[:],
                                       Alu.mult, Alu.subtract)
                vii = keep("vii")
                V.scalar_tensor_tensor(vii[:], s_ii[:], inv_n, mi2[:],
                                       Alu.mult, Alu.subtract)
                vri = keep("vri")
                V.scalar_tensor_tensor(vri[:], s_ri[:], inv_n, mri[:],
                                       Alu.mult, Alu.subtract)

                tau = keep("tau")
                V.tensor_tensor(tau[:], vrr[:], vii[:], Alu.add)
                d1 = scratch("d1")
                V.tensor_tensor(d1[:], vrr[:], vii[:], Alu.mult)
                vri2 = scratch("vri2")
                V.tensor_tensor(vri2[:], vri[:], vri[:], Alu.mult)
                delta = keep("delta")
                V.tensor_tensor(delta[:], d1[:], vri2[:], Alu.subtract)
                V.tensor_scalar_max(delta[:], delta[:], EPS)
                V.tensor_scalar_min(delta[:], delta[:], DELTA_MAX)

                s_t = keep("s_t")
                S.activation(s_t[:], delta[:], Act.Sqrt)
                targ = scratch("targ")
                V.scalar_tensor_tensor(targ[:], s_t[:], 2.0, tau[:],
                                       Alu.mult, Alu.add)
                t_t = keep("t_t")
                S.activation(t_t[:], targ[:], Act.Sqrt)
                st_t = scratch("st_t")
                V.tensor_tensor(st_t[:], s_t[:], t_t[:], Alu.mult)
                rst = keep("rst")
                V.reciprocal(rst[:], st_t[:])

                a1 = scratch("a1")
                V.tensor_tensor(a1[:], s_t[:], vii[:], Alu.add)
                urr = keep("urr")
                V.tensor_tensor(urr[:], a1[:], rst[:], Alu.mult)
                a2 = scratch("a2")
                V.tensor_tensor(a2[:], s_t[:], vrr[:], Alu.add)
                uii = keep("uii")
                V.tensor_tensor(uii[:], a2[:], rst[:], Alu.mult)
                uri = keep("uri")
                V.scalar_tensor_tensor(uri[:], vri[:], -1.0, rst[:],
                                       Alu.mult, Alu.mult)

                def mix(name, wa, ua, wb, ub):
                    g1 = scratch(name + "_g1")
                    V.tensor_tensor(g1[:], wa[:], ua[:], Alu.mult)
                    g2 = scratch(name + "_g2")
                    V.tensor_tensor(g2[:], wb[:], ub[:], Alu.mult)
                    z = keep(name)
                    V.tensor_tensor(z[:], g1[:], g2[:], Alu.add)
                    return z

                zrr = mix("zrr", w_rr, urr, w_ri, uri)
                zri = mix("zri", w_rr, uri, w_ri, uii)
                zir = mix("zir", w_ri, urr, w_ii, uri)
                zii = mix("zii", w_ri, uri, w_ii, uii)

                def bias(name, b0, za, zb_):
                    h1 = scratch(name + "_h1")
                    V.tensor_tensor(h1[:], za[:], mr[:], Alu.mult)
                    h2 = scratch(name + "_h2")
                    V.tensor_tensor(h2[:], zb_[:], mi[:], Alu.mult)
                    h3 = scratch(name + "_h3")
                    V.tensor_tensor(h3[:], h1[:], h2[:], Alu.add)
                    bb = keep(name)
                    V.tensor_tensor(bb[:], b0[:], h3[:], Alu.subtract)
                    return bb

                brp = bias("brp", b_r, zrr, zri)
                bip = bias("bip", b_i, zir, zii)

                # convert the 6 coeff tiles to bf16, gather each into a
                # [1, D] partition-0 row via SBUF->SBUF DMA
                def to_row(coef, row, name):
                    c16 = smp.tile([C, F], bf16, name=name + "16")
                    nc.vector.tensor_copy(c16[:], coef[:])
                    nc.sync.dma_start(out=row[0:1, :], in_=c16[:])

                zrows = [smp.tile([1, D], bf16, name=f"zrow{k}")
                         for k in range(4)]
                for k, coef in enumerate([zrr, zri, zir, zii]):
                    to_row(coef, zrows[k], f"z{k}")
                to_row(brp, brow, "brp")
                to_row(bip, birow, "bip")

                # PE ones-broadcast of the 4 Z rows to [128, D] bf16
                for k in range(4):
                    for ci_, (off, w) in enumerate(ALL_CHUNKS):
                        pb = bps.tile([P, 512], f32, tag="pb",
                                      name=f"pb{k}_{off}")
                        nc.tensor.matmul(pb[:, 0:w], lhsT=ones_row[:],
                                         rhs=zrows[k][0:1, off:off + w],
                                         start=True, stop=True)
                        if (k + ci_) % 2 == 0:
                            nc.scalar.copy(zb[k][:, off:off + w], pb[:, 0:w])
                        else:
                            nc.vector.tensor_copy(zb[k][:, off:off + w],
                                                  pb[:, 0:w])

            # ---------------- phase B: apply from the bf16 cache --------
            with (
                tc.tile_pool(name="prodB", bufs=2) as prod,
                tc.tile_pool(name="outB", bufs=2) as outp,
                tc.tile_pool(name="psB", bufs=2, space="PSUM") as psb,
            ):
                for i in range(T):
                    cr = crs_tiles[i]
                    ci = cis_tiles[i]
                    p1 = prod.tile([P, D], bf16, tag="pa", name=f"p1_{i}")
                    nc.vector.tensor_tensor(p1[:], cr[:], zb[0][:], Alu.mult)
                    p2 = prod.tile([P, D], bf16, tag="pb", name=f"p2_{i}")
                    nc.vector.tensor_tensor(p2[:], ci[:], zb[1][:], Alu.mult)
                    p3 = prod.tile([P, D], bf16, tag="pa", name=f"p3_{i}")
                    nc.vector.tensor_tensor(p3[:], cr[:], zb[2][:], Alu.mult)
                    p4 = prod.tile([P, D], bf16, tag="pb", name=f"p4_{i}")
                    nc.vector.tensor_tensor(p4[:], ci[:], zb[3][:], Alu.mult)

                    yrt = outp.tile([P, D], f32, tag="yr", name=f"yr_{i}")
                    yit = outp.tile([P, D], f32, tag="yi", name=f"yi_{i}")
                    for off, w in ALL_CHUNKS:
                        sl = slice(off, off + w)
                        ps_r = psb.tile([P, 512], f32, tag="pr",
                                        name=f"pr_{i}_{off}")
                        ps_i = psb.tile([P, 512], f32, tag="pi",
                                        name=f"pi_{i}_{off}")
                        # seed with the bias via ones x bias_row outer
                        nc.tensor.matmul(ps_r[:, 0:w], lhsT=ones_row[:],
                                         rhs=brow[0:1, sl],
                                         start=True, stop=False)
                        nc.tensor.matmul(ps_i[:, 0:w], lhsT=ones_row[:],
                                         rhs=birow[0:1, sl],
                                         start=True, stop=False)
                        # identity pass-through adds of the products
                        nc.tensor.matmul(ps_r[:, 0:w], lhsT=ident[:],
                                         rhs=p1[:, sl], start=False,
                                         stop=False)
                        nc.tensor.matmul(ps_r[:, 0:w], lhsT=ident[:],
                                         rhs=p2[:, sl], start=False,
                                         stop=True)
                        nc.tensor.matmul(ps_i[:, 0:w], lhsT=ident[:],
                                         rhs=p3[:, sl], start=False,
                                         stop=False)
                        nc.tensor.matmul(ps_i[:, 0:w], lhsT=ident[:],
                                         rhs=p4[:, sl], start=False,
                                         stop=True)
                        nc.scalar.copy(yrt[:, sl], ps_r[:, 0:w])
                        nc.scalar.copy(yit[:, sl], ps_i[:, 0:w])

                    nc.sync.dma_start(out=yr[i * P:(i + 1) * P, :],
                                      in_=yrt[:])
                    nc.sync.dma_start(out=yi[i * P:(i + 1) * P, :],
                                      in_=yit[:])

            lateps.close()

    nc.compile()
    return nc


def get_nc():
    if "nc" not in _CACHE:
        _CACHE["nc"] = _build()
    return _CACHE["nc"]


def kernel(xr, xi, Wrr, Wri, Wii, Br, Bi):
    from concourse import bass_utils

    nc = get_nc()
    xr2 = np.ascontiguousarray(np.asarray(xr), dtype=np.float32).reshape(N, D)
    xi2 = np.ascontiguousarray(np.asarray(xi), dtype=np.float32).reshape(N, D)
    params = {
        "wrr": np.ascontiguousarray(np.asarray(Wrr), dtype=np.float32),
        "wri": np.ascontiguousarray(np.asarray(Wri), dtype=np.float32),
        "wii": np.ascontiguousarray(np.asarray(Wii), dtype=np.float32),
        "br": np.ascontiguousarray(np.asarray(Br), dtype=np.float32),
        "bi": np.ascontiguousarray(np.asarray(Bi), dtype=np.float32),
    }
    in_maps = []
    for r in range(N_CORES):
        m = {"xr": xr2[r * NS:(r + 1) * NS], "xi": xi2[r * NS:(r + 1) * NS]}
        m.update(params)
        in_maps.append(m)

    res = bass_utils.run_bass_kernel_spmd(nc, in_maps,
                                          core_ids=list(range(N_CORES)))
    yr_ = np.concatenate([res.results[r]["yr"] for r in range(N_CORES)], axis=0)
    yi_ = np.concatenate([res.results[r]["yi"] for r in range(N_CORES)], axis=0)
    return yr_.reshape(N, C, F), yi_.reshape(N, C, F)


# revision 17
# speedup vs baseline: 1.9970x; 1.2512x over previous
"""Trainium2 Bass kernel for complex depthwise batchnorm (training-mode stats).

Data-parallel over batch N across 8 NeuronCores, bf16 on the wire.
Host casts the fp32 inputs to bf16 (error budget 2e-2 vs ~3e-3 incurred);
per core:
  phase A: stream the [2048, 2056] bf16 shard straight into an SBUF cache;
           squares (ACT) / cross+running-sum (DVE) in bf16; per-column sums
           {xr, xi, xr^2, xi^2, xr*xi} via ones-vector matmuls into PSUM
           (the xr sum rides a DVE elementwise accumulator to offload PE).
  AllGather (41KB/rank) of the [5, 2056] partial sums; each core reduces
           the 8 ranks' partials with one selection-matrix matmul.
  coefficient math on [8, 257], then all 6 coefficients (4 Z + 2 bias)
           PE-broadcast to [128, 2056] bf16.
  phase B: DVE forms the 4 products z*x in bf16 2x mode; PE accumulates
           bias+p1+p2 per 512-column group into [128,1024] PSUM tiles with
           a single stationary identity (no LDWEIGHTS swaps); ACT drains
           PSUM -> bf16 SBUF; the 8-column tail is summed on DVE; DMA out
           bf16, host upcasts to fp32.
"""

import numpy as np

N, C, F = 16384, 8, 257
D = C * F  # 2056
N_CORES = 8
NS = N // N_CORES  # 2048
P = 128
T = NS // P  # 16 tiles per core
EPS = 1e-6
DELTA_MAX = 1e8

RED_CHUNKS = [(c * 512, 512) for c in range(4)]
TAIL_OFF, TAIL_W = 2048, D - 2048  # 8 columns

_CACHE = {}


def _build():
    import concourse.bacc as bacc
    import concourse.tile as tile
    import concourse.mybir as mybir

    f32 = mybir.dt.float32
    bf16 = mybir.dt.bfloat16
    Alu = mybir.AluOpType
    Act = mybir.ActivationFunctionType

    nc = bacc.Bacc("TRN2", target_bir_lowering=False, debug=False,
                   num_devices=N_CORES)

    xr = nc.dram_tensor("xr", [NS, D], bf16, kind="ExternalInput").ap()
    xi = nc.dram_tensor("xi", [NS, D], bf16, kind="ExternalInput").ap()
    wrr = nc.dram_tensor("wrr", [C, F], f32, kind="ExternalInput").ap()
    wri = nc.dram_tensor("wri", [C, F], f32, kind="ExternalInput").ap()
    wii = nc.dram_tensor("wii", [C, F], f32, kind="ExternalInput").ap()
    br = nc.dram_tensor("br", [C, F], f32, kind="ExternalInput").ap()
    bi = nc.dram_tensor("bi", [C, F], f32, kind="ExternalInput").ap()
    yr = nc.dram_tensor("yr", [NS, D], bf16, kind="ExternalOutput").ap()
    yi = nc.dram_tensor("yi", [NS, D], bf16, kind="ExternalOutput").ap()

    ALL_CHUNKS = RED_CHUNKS + [(TAIL_OFF, TAIL_W)]

    with tile.TileContext(nc) as tc:
        with (
            tc.tile_pool(name="const", bufs=1) as cpool,
            tc.tile_pool(name="cache", bufs=1) as cachep,
            tc.tile_pool(name="param", bufs=1) as prm,
            tc.tile_pool(name="zb", bufs=1) as zbp,
            tc.tile_pool(name="dram", bufs=1, space="DRAM") as dram,
        ):
            ones_b = cpool.tile([P, 1], bf16, name="ones_b")
            nc.vector.memset(ones_b[:], 1.0)
            ones_row = cpool.tile([1, P], bf16, name="ones_row")
            nc.vector.memset(ones_row[:], 1.0)
            ident = cpool.tile([P, P], bf16, name="ident")
            nc.vector.memset(ident[:], 1.0)
            nc.gpsimd.affine_select(
                out=ident[:], in_=ident[:],
                compare_op=Alu.is_equal, fill=0.0,
                base=0, pattern=[[-1, P]], channel_multiplier=1,
            )
            # [40, 5] rank-fold selection matrix: 8 stacked 5x5 identities
            i5 = cpool.tile([5, 5], f32, name="i5")
            nc.vector.memset(i5[:], 1.0)
            nc.gpsimd.affine_select(
                out=i5[:], in_=i5[:],
                compare_op=Alu.is_equal, fill=0.0,
                base=0, pattern=[[-1, 5]], channel_multiplier=1,
            )
            sel = cpool.tile([40, 5], f32, name="sel")
            for r in range(N_CORES):
                nc.sync.dma_start(out=sel[5 * r:5 * r + 5, :], in_=i5[:])
            # preload the Sqrt activation table off the critical path
            sqwarm = cpool.tile([1, 8], f32, name="sqwarm")
            nc.vector.memset(sqwarm[:], 1.0)
            nc.scalar.activation(sqwarm[:], sqwarm[:], Act.Sqrt)

            crs_tiles = [cachep.tile([P, D], bf16, name=f"cr{i}")
                         for i in range(T)]
            cis_tiles = [cachep.tile([P, D], bf16, name=f"ci{i}")
                         for i in range(T)]

            # params [C, F] loaded up-front (independent of the stats)
            w_rr = prm.tile([C, F], f32, name="w_rr")
            nc.sync.dma_start(out=w_rr[:], in_=wrr[:, :])
            w_ri = prm.tile([C, F], f32, name="w_ri")
            nc.sync.dma_start(out=w_ri[:], in_=wri[:, :])
            w_ii = prm.tile([C, F], f32, name="w_ii")
            nc.sync.dma_start(out=w_ii[:], in_=wii[:, :])
            b_r = prm.tile([C, F], f32, name="b_r")
            nc.sync.dma_start(out=b_r[:], in_=br[:, :])
            b_i = prm.tile([C, F], f32, name="b_i")
            nc.sync.dma_start(out=b_i[:], in_=bi[:, :])

            cc_in = dram.tile([5, D], f32, name="cc_in")
            cc_ag = dram.tile([5 * N_CORES, D], f32, name="cc_ag",
                              addr_space="Shared")
            cc_red = dram.tile([5, D], f32, name="cc_red")

            # ---------------- phase A: local sums + bf16 cache ----------
            #   accA: q0=sum(xr)@p0, q1=sum(xi)@p32, q2=sum(xr^2)@p64
            #   accB: q3=sum(xi^2)@p0, q4=sum(xr*xi)@p32,
            #         all 5 tails (cols 2048:2056) @p64, free offset q*8
            with (
                tc.tile_pool(name="sqA", bufs=2) as sqp,
                tc.tile_pool(name="drn", bufs=1) as drn,
                tc.tile_pool(name="accA", bufs=1, space="PSUM") as accp,
            ):
                accA = accp.tile([65, 2048], f32, name="accA")
                accB = accp.tile([65, 2048], f32, name="accB")
                # running elementwise sum of xr (offloads one PE stream)
                acc_r = drn.tile([P, D], bf16, name="acc_r")
                QSLOT = [(accA, 0), (accA, 32), (accA, 64),
                         (accB, 0), (accB, 32)]

                for i in range(T):
                    cr = crs_tiles[i]
                    nc.sync.dma_start(out=cr[:], in_=xr[i * P:(i + 1) * P, :])
                    ci = cis_tiles[i]
                    nc.sync.dma_start(out=ci[:], in_=xi[i * P:(i + 1) * P, :])

                    sqr = sqp.tile([P, D], bf16, tag="sqr", name=f"sqr_{i}")
                    nc.scalar.activation(sqr[:], cr[:], Act.Square)
                    sqi = sqp.tile([P, D], bf16, tag="sqi", name=f"sqi_{i}")
                    nc.vector.tensor_tensor(sqi[:], ci[:], ci[:], Alu.mult)
                    crs = sqp.tile([P, D], bf16, tag="crs", name=f"crs_{i}")
                    nc.vector.tensor_tensor(crs[:], cr[:], ci[:], Alu.mult)
                    if i == 0:
                        nc.vector.tensor_copy(acc_r[:], cr[:])
                    else:
                        nc.vector.tensor_tensor(acc_r[:], acc_r[:], cr[:],
                                                Alu.add)

                    st, fin = (i == 0), (i == T - 1)
                    RHS = [None, ci, sqr, sqi, crs]
                    for off, w in RED_CHUNKS:
                        sl = slice(off, off + w)
                        for q in range(1, 5):
                            tile_, p = QSLOT[q]
                            nc.tensor.matmul(tile_[p:p + 1, sl],
                                             lhsT=ones_b[:], rhs=RHS[q][:, sl],
                                             start=st, stop=fin)
                    for q in range(1, 5):
                        nc.tensor.matmul(
                            accB[64:65, q * 8:q * 8 + TAIL_W],
                            lhsT=ones_b[:],
                            rhs=RHS[q][:, TAIL_OFF:D],
                            start=(st and q == 1), stop=(fin and q == 4))

                # fold the DVE accumulator into the q0 PSUM slots
                for off, w in RED_CHUNKS:
                    nc.tensor.matmul(accA[0:1, off:off + w], lhsT=ones_b[:],
                                     rhs=acc_r[:, off:off + w],
                                     start=True, stop=True)
                nc.tensor.matmul(accB[64:65, 0:TAIL_W], lhsT=ones_b[:],
                                 rhs=acc_r[:, TAIL_OFF:D],
                                 start=True, stop=True)

                # PSUM -> SBUF rows, then 3 strided gather DMAs to DRAM
                sums_a = drn.tile([P, 2048], f32, name="sums_a")
                sums_b = drn.tile([P, 2048], f32, name="sums_b")
                nc.vector.tensor_copy(sums_a[0:1, :], accA[0:1, :])
                nc.scalar.copy(sums_a[32:33, :], accA[32:33, :])
                nc.vector.tensor_copy(sums_a[64:65, :], accA[64:65, :])
                nc.scalar.copy(sums_b[0:1, :], accB[0:1, :])
                nc.vector.tensor_copy(sums_b[32:33, :], accB[32:33, :])
                nc.scalar.copy(sums_b[64:65, 0:40], accB[64:65, 0:40])

                nc.sync.dma_start(out=cc_in[0:3, 0:2048],
                                  in_=sums_a[0:65:32, :])
                nc.sync.dma_start(out=cc_in[3:5, 0:2048],
                                  in_=sums_b[0:33:32, :])
                for q in range(5):
                    nc.sync.dma_start(
                        out=cc_in[q:q + 1, TAIL_OFF:D],
                        in_=sums_b[64:65, q * 8:q * 8 + TAIL_W])

            # ---------------- all-gather + local rank fold --------------
            nc.gpsimd.collective_compute(
                "AllGather",
                Alu.bypass,
                replica_groups=[list(range(N_CORES))],
                ins=[cc_in[:].opt()],
                outs=[cc_ag[:].opt()],
            )

            # 6 broadcast coefficient tiles persist into phase B
            zbt = [zbp.tile([P, D], bf16, name=f"zb{k}") for k in range(6)]

            with tc.tile_pool(name="mid", bufs=1) as smp:
                with (
                    tc.tile_pool(name="agp", bufs=1) as agp,
                    tc.tile_pool(name="ps5p", bufs=1, space="PSUM") as ps5p,
                ):
                    ag_sb = agp.tile([5 * N_CORES, D], f32, name="ag_sb")
                    nc.sync.dma_start(out=ag_sb[:], in_=cc_ag[:])
                    ps5 = ps5p.tile([5, 2048], f32, name="ps5")
                    ps5t = ps5p.tile([5, 512], f32, name="ps5t")
                    for off, w in RED_CHUNKS:
                        nc.tensor.matmul(ps5[0:5, off:off + w], lhsT=sel[:],
                                         rhs=ag_sb[:, off:off + w],
                                         start=True, stop=True)
                    nc.tensor.matmul(ps5t[0:5, 0:TAIL_W], lhsT=sel[:],
                                     rhs=ag_sb[:, TAIL_OFF:D],
                                     start=True, stop=True)
                    sums_sb = smp.tile([5, D], f32, name="sums_sb")
                    nc.vector.tensor_copy(sums_sb[0:5, 0:2048], ps5[:, :])
                    nc.scalar.copy(sums_sb[0:5, TAIL_OFF:D], ps5t[0:5, 0:8])
                    # bounce through DRAM: a [1, D] -> [C, F] partition
                    # spread is an illegal SBUF->SBUF access pattern
                    nc.sync.dma_start(out=cc_red[:], in_=sums_sb[:])
                    red_cf = cc_red[:].rearrange("q (c f) -> (q c) f", c=C)

                    def spread(name, q):
                        t = smp.tile([C, F], f32, name=name)
                        nc.sync.dma_start(out=t[:],
                                          in_=red_cf[q * C:(q + 1) * C, :])
                        return t

                    s_xr = spread("s_xr", 0)
                    s_xi = spread("s_xi", 1)
                    s_rr = spread("s_rr", 2)
                    s_ii = spread("s_ii", 3)
                    s_ri = spread("s_ri", 4)

                ctp = lateps_mid = None  # replaced below
                from contextlib import ExitStack
                lateps_mid = ExitStack()
                ctp = lateps_mid.enter_context(
                    tc.tile_pool(name="mids", bufs=4))
                rowp = lateps_mid.enter_context(
                    tc.tile_pool(name="rowp", bufs=2))
                bps = lateps_mid.enter_context(
                    tc.tile_pool(name="bps", bufs=4, space="PSUM"))

                inv_n = 1.0 / N
                V = nc.vector
                S = nc.scalar

                def keep(name):
                    return smp.tile([C, F], f32, name=name)

                def scratch(name):
                    return ctp.tile([C, F], f32, tag="ct", name=name)

                # means (in place over the sums)
                mr = s_xr
                V.tensor_scalar_mul(mr[:], s_xr[:], inv_n)
                mi = s_xi
                V.tensor_scalar_mul(mi[:], s_xi[:], inv_n)

                mr2 = scratch("mr2")
                V.tensor_tensor(mr2[:], mr[:], mr[:], Alu.mult)
                mi2 = scratch("mi2")
                V.tensor_tensor(mi2[:], mi[:], mi[:], Alu.mult)
                mri = scratch("mri")
                V.tensor_tensor(mri[:], mr[:], mi[:], Alu.mult)

                vrr = s_rr
                V.scalar_tensor_tensor(vrr[:], s_rr[:], inv_n, mr2[:],
                                       Alu.mult, Alu.subtract)
                vii = s_ii
                V.scalar_tensor_tensor(vii[:], s_ii[:], inv_n, mi2[:],
                                       Alu.mult, Alu.subtract)
                vri = s_ri
                V.scalar_tensor_tensor(vri[:], s_ri[:], inv_n, mri[:],
                                       Alu.mult, Alu.subtract)

                tau = keep("tau")
                V.tensor_tensor(tau[:], vrr[:], vii[:], Alu.add)
                d1 = scratch("d1")
                V.tensor_tensor(d1[:], vrr[:], vii[:], Alu.mult)
                vri2 = scratch("vri2")
                V.tensor_tensor(vri2[:], vri[:], vri[:], Alu.mult)
                delta = keep("delta")
                V.tensor_tensor(delta[:], d1[:], vri2[:], Alu.subtract)
                V.tensor_scalar_max(delta[:], delta[:], EPS)
                V.tensor_scalar_min(delta[:], delta[:], DELTA_MAX)

                s_t = keep("s_t")
                S.activation(s_t[:], delta[:], Act.Sqrt)
                targ = scratch("targ")
                V.scalar_tensor_tensor(targ[:], s_t[:], 2.0, tau[:],
                                       Alu.mult, Alu.add)
                # rst = 1 / (sqrt(delta) * sqrt(targ)) = 1 / sqrt(delta*targ)
                dm = scratch("dm")
                V.tensor_tensor(dm[:], delta[:], targ[:], Alu.mult)
                sm = scratch("sm")
                S.activation(sm[:], dm[:], Act.Sqrt)
                rst = keep("rst")
                V.reciprocal(rst[:], sm[:])

                a1 = scratch("a1")
                V.tensor_tensor(a1[:], s_t[:], vii[:], Alu.add)
                urr = keep("urr")
                V.tensor_tensor(urr[:], a1[:], rst[:], Alu.mult)
                a2 = scratch("a2")
                V.tensor_tensor(a2[:], s_t[:], vrr[:], Alu.add)
                uii = keep("uii")
                V.tensor_tensor(uii[:], a2[:], rst[:], Alu.mult)
                uri = keep("uri")
                V.scalar_tensor_tensor(uri[:], vri[:], -1.0, rst[:],
                                       Alu.mult, Alu.mult)

                def mix(name, wa, ua, wb, ub):
                    g1 = scratch(name + "_g1")
                    V.tensor_tensor(g1[:], wa[:], ua[:], Alu.mult)
                    g2 = scratch(name + "_g2")
                    V.tensor_tensor(g2[:], wb[:], ub[:], Alu.mult)
                    z = keep(name)
                    V.tensor_tensor(z[:], g1[:], g2[:], Alu.add)
                    return z

                zrr = mix("zrr", w_rr, urr, w_ri, uri)
                zri = mix("zri", w_rr, uri, w_ri, uii)
                zir = mix("zir", w_ri, urr, w_ii, uri)
                zii = mix("zii", w_ri, uri, w_ii, uii)

                def bias_cf(name, b0, za, zb_):
                    h1 = scratch(name + "_h1")
                    V.tensor_tensor(h1[:], za[:], mr[:], Alu.mult)
                    h2 = scratch(name + "_h2")
                    V.tensor_tensor(h2[:], zb_[:], mi[:], Alu.mult)
                    h3 = scratch(name + "_h3")
                    V.tensor_tensor(h3[:], h1[:], h2[:], Alu.add)
                    bb = keep(name)
                    V.tensor_tensor(bb[:], b0[:], h3[:], Alu.subtract)
                    return bb

                brp = bias_cf("brp", b_r, zrr, zri)
                bip = bias_cf("bip", b_i, zir, zii)

                # bf16-convert + row-gather + PE ones-broadcast, one at a time
                for k, coef in enumerate([zrr, zri, zir, zii, brp, bip]):
                    c16 = smp.tile([C, F], bf16, name=f"c16_{k}")
                    nc.vector.tensor_copy(c16[:], coef[:])
                    row = rowp.tile([1, D], bf16, tag="row", name=f"row{k}")
                    nc.sync.dma_start(out=row[0:1, :], in_=c16[:])
                    for ci_, (off, w) in enumerate(ALL_CHUNKS):
                        pb = bps.tile([P, 512], f32, tag="pb",
                                      name=f"pb{k}_{off}")
                        nc.tensor.matmul(pb[:, 0:w], lhsT=ones_row[:],
                                         rhs=row[0:1, off:off + w],
                                         start=True, stop=True)
                        if (k + ci_) % 2 == 0:
                            nc.scalar.copy(zbt[k][:, off:off + w], pb[:, 0:w])
                        else:
                            nc.vector.tensor_copy(zbt[k][:, off:off + w],
                                                  pb[:, 0:w])
                lateps_mid.close()

            # ---------------- phase B: apply from the bf16 cache --------
            # groups per [128,1024] PSUM tile: two 512-col accumulations of
            # (bias, p_a, p_b); everything streams through the stationary
            # identity so the PE never swaps weights.
            zb_rr, zb_ri, zb_ir, zb_ii, bbr, bbi = zbt
            with (
                tc.tile_pool(name="prodB", bufs=3) as prod,
                tc.tile_pool(name="outB", bufs=2) as outp,
                tc.tile_pool(name="psB", bufs=2, space="PSUM") as psb,
            ):
                for i in range(T):
                    cr = crs_tiles[i]
                    ci = cis_tiles[i]
                    p1 = prod.tile([P, D], bf16, tag="pa", name=f"p1_{i}")
                    nc.vector.tensor_tensor(p1[:], cr[:], zb_rr[:], Alu.mult)
                    p2 = prod.tile([P, D], bf16, tag="pb", name=f"p2_{i}")
                    nc.vector.tensor_tensor(p2[:], ci[:], zb_ri[:], Alu.mult)
                    p3 = prod.tile([P, D], bf16, tag="pa", name=f"p3_{i}")
                    nc.vector.tensor_tensor(p3[:], cr[:], zb_ir[:], Alu.mult)
                    p4 = prod.tile([P, D], bf16, tag="pb", name=f"p4_{i}")
                    nc.vector.tensor_tensor(p4[:], ci[:], zb_ii[:], Alu.mult)

                    yrt = outp.tile([P, D], bf16, tag="yr", name=f"yr_{i}")
                    yit = outp.tile([P, D], bf16, tag="yi", name=f"yi_{i}")
                    for out_t, pa, pb_, bias_t, ptag in (
                        (yrt, p1, p2, bbr, "pr"),
                        (yit, p3, p4, bbi, "pi"),
                    ):
                        for half in range(2):
                            ps = psb.tile([P, 1024], f32, tag=ptag,
                                          name=f"{ptag}_{i}_{half}")
                            for cpos in range(2):
                                off = half * 1024 + cpos * 512
                                sl = slice(off, off + 512)
                                psl = slice(cpos * 512, cpos * 512 + 512)
                                nc.tensor.matmul(ps[:, psl], lhsT=ident[:],
                                                 rhs=bias_t[:, sl],
                                                 start=True, stop=False)
                                nc.tensor.matmul(ps[:, psl], lhsT=ident[:],
                                                 rhs=pa[:, sl],
                                                 start=False, stop=False)
                                nc.tensor.matmul(ps[:, psl], lhsT=ident[:],
                                                 rhs=pb_[:, sl],
                                                 start=False, stop=True)
                            nc.scalar.copy(
                                out_t[:, half * 1024:half * 1024 + 1024],
                                ps[:, :])
                        # 8-column tail on DVE straight from the products
                        tsl = slice(TAIL_OFF, D)
                        nc.vector.tensor_tensor(out_t[:, tsl], pa[:, tsl],
                                                pb_[:, tsl], Alu.add)
                        nc.vector.tensor_tensor(out_t[:, tsl], out_t[:, tsl],
                                                bias_t[:, tsl], Alu.add)

                    nc.sync.dma_start(out=yr[i * P:(i + 1) * P, :],
                                      in_=yrt[:])
                    nc.sync.dma_start(out=yi[i * P:(i + 1) * P, :],
                                      in_=yit[:])

    nc.compile()
    return nc


def get_nc():
    if "nc" not in _CACHE:
        _CACHE["nc"] = _build()
    return _CACHE["nc"]


def make_in_maps(xr, xi, Wrr, Wri, Wii, Br, Bi):
    import ml_dtypes

    bf = ml_dtypes.bfloat16
    xr2 = np.ascontiguousarray(np.asarray(xr).reshape(N, D).astype(bf))
    xi2 = np.ascontiguousarray(np.asarray(xi).reshape(N, D).astype(bf))
    params = {
        "wrr": np.ascontiguousarray(np.asarray(Wrr), dtype=np.float32),
        "wri": np.ascontiguousarray(np.asarray(Wri), dtype=np.float32),
        "wii": np.ascontiguousarray(np.asarray(Wii), dtype=np.float32),
        "br": np.ascontiguousarray(np.asarray(Br), dtype=np.float32),
        "bi": np.ascontiguousarray(np.asarray(Bi), dtype=np.float32),
    }
    in_maps = []
    for r in range(N_CORES):
        m = {"xr": xr2[r * NS:(r + 1) * NS], "xi": xi2[r * NS:(r + 1) * NS]}
        m.update(params)
        in_maps.append(m)
    return in_maps


def kernel(xr, xi, Wrr, Wri, Wii, Br, Bi):
    from concourse import bass_utils

    nc = get_nc()
    in_maps = make_in_maps(xr, xi, Wrr, Wri, Wii, Br, Bi)

    res = bass_utils.run_bass_kernel_spmd(nc, in_maps,
                                          core_ids=list(range(N_CORES)))
    yr_ = np.concatenate(
        [np.asarray(res.results[r]["yr"]).astype(np.float32)
         for r in range(N_CORES)], axis=0)
    yi_ = np.concatenate(
        [np.asarray(res.results[r]["yi"]).astype(np.float32)
         for r in range(N_CORES)], axis=0)
    return yr_.reshape(N, C, F), yi_.reshape(N, C, F)
